# revision 28
# baseline (speedup 1.0000x reference)
"""Trainium2 Bass kernel for low-rank shared-QK attention.

Reference computation (per batch element b of 8):
    xQ     = x[b] @ (Q / sqrt(D))            # [S, R]
    scores = softmax(xQ @ xQ^T, axis=-1)     # [S, S]
    y[b]   = scores @ x[b]                   # [S, D]

with S=4096, D=1024, R=64, B=8. Pure data parallel: one batch element
per NeuronCore (8 cores).

Per-core kernel strategy:
  Phase A: DMA x into SBUF staging tiles; PE-transpose 128x128 blocks
    to build xT tiles; MM1 computes T = (x @ Qs)^T into SBUF
    [128, 4096] (rows 64..127 zero via zero-padded Qs columns). In
    parallel, ACT rounds x into the resident f32r x_sb [128, 32, 1024].
  Main loop (logits are symmetric: L = T^T T), software-pipelined two
  n-iterations ahead so ACT's exp overlaps the PE's PV matmuls:
    for each m-group (256 query rows), for each n-chunk (128 key rows):
      Lt[n, m]  = matmul(lhsT=T[:, n], rhs=T[:, m])       (PSUM)
      Et[n, m]  = exp(Lt)                                 (ACT, -> SBUF)
      y_psum   += Et.T @ x[n]            (MM3, accumulate over n)
      acc      += Et                     (row-sum accumulate, on DVE)
    rowsum[m] = reduce(transpose(acc))   (PE transpose + DVE reduce)
    y[m] = y_psum * (1 / rowsum)         (per-partition scale, DMA out)
  Row sums run off the PE (DVE accumulate + one transpose per m-block)
  because every extra matmul costs a ~188 ns fp32r weight load; the PE
  stays on the streaming floor (1 column/cycle).
  No max-subtraction in softmax: logits are O(1) here (|L| < ~4), and
  exp is computed in fp32. All matmuls run as float32r (TF32-mode,
  1 cyc/row at N>=256) with fp32 PSUM accumulation. The BIR verifier
  requires f32r matmul operands to be *produced* as f32r (rounding
  happens in the producing engine), hence the f32r-dtyped tiles and
  rounding copies.
"""

import numpy as np

S = 4096
D = 1024
R = 64
B = 8
P = 128
SC = S // P  # 32 s-chunks
DC = D // P  # 8 d-chunks
SG = 256     # phase-A s-group (2 chunks)
MG = 256     # main-loop m-group
NMG = S // MG


def build_bass():
    import concourse.bacc as bacc
    import concourse.mybir as mybir
    import concourse.tile as tile
    from concourse.masks import make_identity

    f32 = mybir.dt.float32
    f32r = mybir.dt.float32r

    nc = bacc.Bacc("TRN2", target_bir_lowering=False, debug=False)
    x_d = nc.dram_tensor("x", [S, D], f32, kind="ExternalInput").ap()
    q_d = nc.dram_tensor("q", [D, R], f32, kind="ExternalInput").ap()
    y_d = nc.dram_tensor("y", [S, D], f32, kind="ExternalOutput").ap()

    with tile.TileContext(nc) as tc:
        # ---- persistent pools ----
        with (
            tc.tile_pool(name="const", bufs=1) as cpool,
            tc.tile_pool(name="xres", bufs=1) as xpool,
            tc.tile_pool(name="tres", bufs=1) as tpool,
        ):
            ident = cpool.tile([P, P], f32, name="ident")
            make_identity(nc, ident)
            qs = cpool.tile([P, DC, P], f32r, name="qs")

            x_sb = xpool.tile([P, SC, D], f32r, name="x_sb")
            T_sb = tpool.tile([P, S], f32r, name="T_sb")

            # ---- phase A: load x, transpose, compute T = (x @ Qs)^T ----
            with (
                tc.tile_pool(name="pa_sbuf", bufs=2) as pa_pool,
                tc.tile_pool(name="pa_stage", bufs=6) as pa_stage,
                tc.tile_pool(name="pa_psum", bufs=2, space="PSUM") as pa_psum,
                tc.tile_pool(name="pa_tpsum", bufs=2, space="PSUM") as pa_tpsum,
            ):
                # qs padded to M=128 (cols R..127 zero) so MM1 writes all
                # 128 partitions of T and T needs no separate zeroing.
                qs_stage = pa_stage.tile([P, DC, P], f32, name="qs_stage", bufs=1)
                nc.vector.memset(qs_stage, 0.0)
                nc.sync.dma_start(
                    qs_stage[:, :, :R], q_d.rearrange("(dc p) r -> p dc r", p=P)
                )
                nc.vector.tensor_copy(qs[:], qs_stage[:])

                for g in range(S // SG):
                    stages = []
                    for s4 in range(SG // P):
                        sc = g * (SG // P) + s4
                        xstage = pa_stage.tile([P, D], f32, name="xstage")
                        nc.sync.dma_start(xstage[:], x_d[sc * P : (sc + 1) * P, :])
                        # off the critical path: ACT rounds x to f32r for MM3
                        nc.scalar.copy(x_sb[:, sc, :], xstage[:])
                        stages.append(xstage)
                    xT = pa_pool.tile([P, DC, SG], f32r, name="xT")
                    for dc in range(DC):
                        xTp = pa_psum.tile([P, SG], f32, name="xTp")
                        for s4 in range(SG // P):
                            nc.tensor.matmul(
                                xTp[:, s4 * P : (s4 + 1) * P],
                                stages[s4][:, dc * P : (dc + 1) * P],
                                ident,
                                is_transpose=True,
                                start=(s4 == 0),
                                stop=(s4 == SG // P - 1),
                            )
                        nc.vector.tensor_copy(xT[:, dc, :], xTp[:])
                    Tp = pa_tpsum.tile([P, SG], f32, name="Tp")
                    for dc in range(DC):
                        nc.tensor.matmul(
                            Tp[:],
                            qs[:, dc, :],
                            xT[:, dc, :],
                            start=(dc == 0),
                            stop=(dc == DC - 1),
                        )
                    nc.scalar.copy(T_sb[:, g * SG : (g + 1) * SG], Tp[:])

            # ---- main loop (flat over (gm, n), pipelined 2 iters ahead) ----
            with (
                tc.tile_pool(name="mn_sbuf", bufs=3) as mn_pool,
                tc.tile_pool(name="y_sbuf", bufs=3) as y_pool,
                tc.tile_pool(name="mn_psum", bufs=1, space="PSUM") as mn_psum,
                tc.tile_pool(name="lt_psum", bufs=2, space="PSUM") as lt_psum,
            ):
                NIT = NMG * SC
                ets = {}

                def mm2_exp(k):
                    gm, n = divmod(k, SC)
                    m0 = gm * MG
                    ltp = lt_psum.tile([P, MG], f32, name="ltp", bufs=3)
                    nc.tensor.matmul(
                        ltp[:],
                        T_sb[:, n * P : (n + 1) * P],
                        T_sb[:, m0 : m0 + MG],
                        start=True,
                        stop=True,
                    )
                    et = mn_pool.tile([P, MG], f32r, name="et", bufs=4)
                    nc.scalar.activation(
                        et[:], ltp[:], mybir.ActivationFunctionType.Exp
                    )
                    ets[k] = et

                mm2_exp(0)
                mm2_exp(1)
                yp = acc = None
                for k in range(NIT):
                    gm, n = divmod(k, SC)
                    m0 = gm * MG
                    if k + 2 < NIT:
                        mm2_exp(k + 2)
                    if n == 0:
                        yp = [
                            [
                                mn_psum.tile([P, 512], f32, name=f"yp_{mb}_{dh}")
                                for dh in range(2)
                            ]
                            for mb in range(2)
                        ]
                        acc = mn_pool.tile([P, MG], f32, name="acc", bufs=2)
                    et = ets.pop(k)
                    for mb in range(2):
                        lhsT = et[:, mb * P : (mb + 1) * P]
                        for dh in range(2):
                            nc.tensor.matmul(
                                yp[mb][dh][:],
                                lhsT,
                                x_sb[:, n, dh * 512 : (dh + 1) * 512],
                                start=(n == 0),
                                stop=(n == SC - 1),
                            )
                    # row-sum accumulation off the PE: acc += et on DVE
                    if n == 0:
                        nc.vector.tensor_copy(acc[:], et[:])
                    else:
                        nc.vector.tensor_add(acc[:], acc[:], et[:])
                    if n == SC - 1:
                        # drain PSUM first (plain copies) so the next
                        # m-group's accumulating matmuls aren't blocked on
                        # the normalize chain; normalize in SBUF after.
                        y_sbs = []
                        for mb in range(2):
                            y_sb = y_pool.tile([P, D], f32, name="y_sb")
                            for dh in range(2):
                                nc.vector.tensor_copy(
                                    y_sb[:, dh * 512 : (dh + 1) * 512],
                                    yp[mb][dh][:],
                                )
                            y_sbs.append(y_sb)
                        for mb in range(2):
                            # acc holds colsums in [n-part, m]; transpose the
                            # mb block on the PE, reduce along free -> [m, 1]
                            accT = lt_psum.tile([P, P], f32, name="accT", bufs=1)
                            nc.tensor.matmul(
                                accT[:],
                                acc[:, mb * P : (mb + 1) * P],
                                ident,
                                is_transpose=True,
                                start=True,
                                stop=True,
                            )
                            rsum = mn_pool.tile([P, 1], f32, name="rsum")
                            nc.vector.reduce_sum(
                                rsum[:], accT[:], axis=mybir.AxisListType.X
                            )
                            inv = mn_pool.tile([P, 1], f32, name="inv")
                            nc.vector.reciprocal(inv[:], rsum[:])
                            y_sb = y_sbs[mb]
                            nc.vector.tensor_scalar_mul(y_sb[:], y_sb[:], inv[:])
                            r0 = m0 + mb * P
                            nc.sync.dma_start(y_d[r0 : r0 + P, :], y_sb[:])

    nc.compile()
    return nc


_NC_CACHE = None


def _get_nc():
    global _NC_CACHE
    if _NC_CACHE is None:
        _NC_CACHE = build_bass()
    return _NC_CACHE


def kernel(x: np.ndarray, Q: np.ndarray) -> np.ndarray:
    from concourse.bass_utils import run_bass_kernel_spmd

    x = np.asarray(x, dtype=np.float32)
    Q = np.asarray(Q, dtype=np.float32)
    assert x.shape == (B, S, D) and Q.shape == (D, R)
    qs = (Q * np.float32(1.0 / np.sqrt(D))).astype(np.float32)
    in_maps = [
        {"x": np.ascontiguousarray(x[b], dtype=np.float32), "q": qs} for b in range(B)
    ]
    nc = _get_nc()
    res = run_bass_kernel_spmd(nc, in_maps, core_ids=list(range(B)))
    out = np.stack([res.results[b]["y"] for b in range(B)], axis=0)
    return out.astype(np.float32)


# revision 29
# speedup vs baseline: 1.2136x; 1.2136x over previous
"""Trainium2 Bass kernel for low-rank shared-QK attention.

Reference computation (per batch element b of 8):
    xQ     = x[b] @ (Q / sqrt(D))            # [S, R]
    scores = softmax(xQ @ xQ^T, axis=-1)     # [S, S]
    y[b]   = scores @ x[b]                   # [S, D]

with S=4096, D=1024, R=64, B=8. Pure data parallel: one batch element
per NeuronCore (8 cores).

Per-core kernel strategy:
  Phase A: DMA x into SBUF staging tiles; PE-transpose 128x128 blocks
    to build xT tiles; MM1 computes T = (x @ Qs)^T into SBUF
    [128, 4096] (rows 64..127 zero via zero-padded Qs columns). In
    parallel, ACT rounds x into the resident f32r x_sb [128, 32, 1024].
  Main loop (logits are symmetric: L = T^T T), software-pipelined two
  n-iterations ahead so ACT's exp overlaps the PE's PV matmuls:
    for each m-group (256 query rows), for each n-chunk (128 key rows):
      Lt[n, m]  = matmul(lhsT=T[:, n], rhs=T[:, m])       (PSUM)
      Et[n, m]  = exp(Lt)                                 (ACT, -> SBUF)
      y_psum   += Et.T @ x[n]            (MM3, accumulate over n)
      acc      += Et                     (row-sum accumulate, on DVE)
    rowsum[m] = reduce(transpose(acc))   (PE transpose + DVE reduce)
    y[m] = y_psum * (1 / rowsum)         (per-partition scale, DMA out)
  Row sums run off the PE (DVE accumulate + one transpose per m-block)
  because every extra matmul costs a ~188 ns fp32r weight load; the PE
  stays on the streaming floor (1 column/cycle).
  No max-subtraction in softmax: logits are O(1) here (|L| < ~4), and
  exp is computed in fp32. All matmuls run as float32r (TF32-mode,
  1 cyc/row at N>=256) with fp32 PSUM accumulation. The BIR verifier
  requires f32r matmul operands to be *produced* as f32r (rounding
  happens in the producing engine), hence the f32r-dtyped tiles and
  rounding copies.
"""

import numpy as np

S = 4096
D = 1024
R = 64
B = 8
P = 128
SC = S // P  # 32 s-chunks
DC = D // P  # 8 d-chunks
SG = 256     # phase-A s-group (2 chunks)
MG = 256     # main-loop m-group
NMG = S // MG


def build_bass():
    import concourse.bacc as bacc
    import concourse.mybir as mybir
    import concourse.tile as tile
    from concourse.masks import make_identity

    f32 = mybir.dt.float32
    f32r = mybir.dt.float32r

    nc = bacc.Bacc("TRN2", target_bir_lowering=False, debug=False)
    x_d = nc.dram_tensor("x", [S, D], f32, kind="ExternalInput").ap()
    q_d = nc.dram_tensor("q", [D, R], f32, kind="ExternalInput").ap()
    y_d = nc.dram_tensor("y", [S, D], f32, kind="ExternalOutput").ap()

    with tile.TileContext(nc) as tc:
        # ---- persistent pools ----
        with (
            tc.tile_pool(name="const", bufs=1) as cpool,
            tc.tile_pool(name="xres", bufs=1) as xpool,
            tc.tile_pool(name="tres", bufs=1) as tpool,
        ):
            ident = cpool.tile([P, P], f32, name="ident")
            make_identity(nc, ident)
            qs = cpool.tile([P, DC, P], f32r, name="qs")

            x_sb = xpool.tile([P, SC, D], f32r, name="x_sb")
            T_sb = tpool.tile([P, S], f32r, name="T_sb")

            # ---- phase A: load x, transpose, compute T = (x @ Qs)^T ----
            with (
                tc.tile_pool(name="pa_sbuf", bufs=2) as pa_pool,
                tc.tile_pool(name="pa_stage", bufs=7) as pa_stage,
                tc.tile_pool(name="pa_psum", bufs=3, space="PSUM") as pa_psum,
                tc.tile_pool(name="pa_tpsum", bufs=2, space="PSUM") as pa_tpsum,
            ):
                # qs padded to M=128 (cols R..127 zero) so MM1 writes all
                # 128 partitions of T and T needs no separate zeroing.
                qs_stage = pa_stage.tile([P, DC, P], f32, name="qs_stage", bufs=1)
                nc.vector.memset(qs_stage, 0.0)
                nc.sync.dma_start(
                    qs_stage[:, :, :R], q_d.rearrange("(dc p) r -> p dc r", p=P)
                )
                nc.vector.tensor_copy(qs[:], qs_stage[:])

                for g in range(S // SG):
                    stages = []
                    for s4 in range(SG // P):
                        sc = g * (SG // P) + s4
                        xstage = pa_stage.tile([P, D], f32, name="xstage")
                        nc.sync.dma_start(xstage[:], x_d[sc * P : (sc + 1) * P, :])
                        # off the critical path: ACT rounds x to f32r for MM3
                        nc.scalar.copy(x_sb[:, sc, :], xstage[:])
                        stages.append(xstage)
                    xT = pa_pool.tile([P, DC, SG], f32r, name="xT")
                    for dc in range(DC):
                        xTp = pa_psum.tile([P, SG], f32, name="xTp")
                        for s4 in range(SG // P):
                            nc.tensor.matmul(
                                xTp[:, s4 * P : (s4 + 1) * P],
                                stages[s4][:, dc * P : (dc + 1) * P],
                                ident,
                                is_transpose=True,
                                start=(s4 == 0),
                                stop=(s4 == SG // P - 1),
                            )
                        nc.vector.tensor_copy(xT[:, dc, :], xTp[:])
                    Tp = pa_tpsum.tile([P, SG], f32, name="Tp")
                    for dc in range(DC):
                        nc.tensor.matmul(
                            Tp[:],
                            qs[:, dc, :],
                            xT[:, dc, :],
                            start=(dc == 0),
                            stop=(dc == DC - 1),
                        )
                    nc.scalar.copy(T_sb[:, g * SG : (g + 1) * SG], Tp[:])

            # ---- main loop (flat over (gm, n), pipelined 2 iters ahead) ----
            with (
                tc.tile_pool(name="mn_sbuf", bufs=3) as mn_pool,
                tc.tile_pool(name="y_sbuf", bufs=3) as y_pool,
                tc.tile_pool(name="mn_psum", bufs=1, space="PSUM") as mn_psum,
                tc.tile_pool(name="lt_psum", bufs=2, space="PSUM") as lt_psum,
            ):
                NIT = NMG * SC
                ets = {}

                def mm2_exp(k):
                    gm, n = divmod(k, SC)
                    m0 = gm * MG
                    ltp = lt_psum.tile([P, MG], f32, name="ltp", bufs=3)
                    nc.tensor.matmul(
                        ltp[:],
                        T_sb[:, n * P : (n + 1) * P],
                        T_sb[:, m0 : m0 + MG],
                        start=True,
                        stop=True,
                    )
                    et = mn_pool.tile([P, MG], f32r, name="et", bufs=4)
                    nc.scalar.activation(
                        et[:], ltp[:], mybir.ActivationFunctionType.Exp
                    )
                    ets[k] = et

                mm2_exp(0)
                mm2_exp(1)
                yp = acc = None
                for k in range(NIT):
                    gm, n = divmod(k, SC)
                    m0 = gm * MG
                    if k + 2 < NIT:
                        mm2_exp(k + 2)
                    if n == 0:
                        yp = [
                            [
                                mn_psum.tile([P, 512], f32, name=f"yp_{mb}_{dh}")
                                for dh in range(2)
                            ]
                            for mb in range(2)
                        ]
                        acc = mn_pool.tile([P, MG], f32, name="acc", bufs=2)
                    et = ets.pop(k)
                    for mb in range(2):
                        lhsT = et[:, mb * P : (mb + 1) * P]
                        for dh in range(2):
                            nc.tensor.matmul(
                                yp[mb][dh][:],
                                lhsT,
                                x_sb[:, n, dh * 512 : (dh + 1) * 512],
                                start=(n == 0),
                                stop=(n == SC - 1),
                            )
                    # row-sum accumulation off the PE: acc += et on DVE
                    if n == 0:
                        nc.vector.tensor_copy(acc[:], et[:])
                    else:
                        nc.vector.tensor_add(acc[:], acc[:], et[:])
                    if n == SC - 1:
                        # drain PSUM first (plain copies) so the next
                        # m-group's accumulating matmuls aren't blocked on
                        # the normalize chain; normalize in SBUF after.
                        y_sbs = []
                        for mb in range(2):
                            y_sb = y_pool.tile([P, D], f32, name="y_sb")
                            for dh in range(2):
                                nc.vector.tensor_copy(
                                    y_sb[:, dh * 512 : (dh + 1) * 512],
                                    yp[mb][dh][:],
                                )
                            y_sbs.append(y_sb)
                        for mb in range(2):
                            # acc holds colsums in [n-part, m]; transpose the
                            # mb block on the PE, reduce along free -> [m, 1]
                            accT = lt_psum.tile([P, P], f32, name="accT", bufs=1)
                            nc.tensor.matmul(
                                accT[:],
                                acc[:, mb * P : (mb + 1) * P],
                                ident,
                                is_transpose=True,
                                start=True,
                                stop=True,
                            )
                            rsum = mn_pool.tile([P, 1], f32, name="rsum")
                            nc.vector.reduce_sum(
                                rsum[:], accT[:], axis=mybir.AxisListType.X
                            )
                            inv = mn_pool.tile([P, 1], f32, name="inv")
                            nc.vector.reciprocal(inv[:], rsum[:])
                            y_sb = y_sbs[mb]
                            nc.vector.tensor_scalar_mul(y_sb[:], y_sb[:], inv[:])
                            r0 = m0 + mb * P
                            nc.sync.dma_start(y_d[r0 : r0 + P, :], y_sb[:])

    nc.compile()
    return nc


_NC_CACHE = None


def _get_nc():
    global _NC_CACHE
    if _NC_CACHE is None:
        _NC_CACHE = build_bass()
    return _NC_CACHE


def kernel(x: np.ndarray, Q: np.ndarray) -> np.ndarray:
    from concourse.bass_utils import run_bass_kernel_spmd

    x = np.asarray(x, dtype=np.float32)
    Q = np.asarray(Q, dtype=np.float32)
    assert x.shape == (B, S, D) and Q.shape == (D, R)
    qs = (Q * np.float32(1.0 / np.sqrt(D))).astype(np.float32)
    in_maps = [
        {"x": np.ascontiguousarray(x[b], dtype=np.float32), "q": qs} for b in range(B)
    ]
    nc = _get_nc()
    res = run_bass_kernel_spmd(nc, in_maps, core_ids=list(range(B)))
    out = np.stack([res.results[b]["y"] for b in range(B)], axis=0)
    return out.astype(np.float32)


# revision 31
# speedup vs baseline: 1.2442x; 1.0252x over previous
"""Trainium2 Bass kernel for low-rank shared-QK attention.

Reference computation (per batch element b of 8):
    xQ     = x[b] @ (Q / sqrt(D))            # [S, R]
    scores = softmax(xQ @ xQ^T, axis=-1)     # [S, S]
    y[b]   = scores @ x[b]                   # [S, D]

with S=4096, D=1024, R=64, B=8. Pure data parallel: one batch element
per NeuronCore (8 cores).

Per-core kernel strategy:
  Phase A: DMA x into SBUF staging tiles; PE-transpose 128x128 blocks
    to build xT tiles; MM1 computes T = (x @ Qs)^T into SBUF
    [128, 4096] (rows 64..127 zero via zero-padded Qs columns). In
    parallel, ACT rounds x into the resident f32r x_sb [128, 32, 1024].
  Main loop (logits are symmetric: L = T^T T), software-pipelined two
  n-iterations ahead so ACT's exp overlaps the PE's PV matmuls, and
  m-groups processed in pairs (512-wide MM2, B-half exp-scores parked
  in a resident SBUF buffer so the odd group's n-loop needs no MM2):
    for each m-group (256 query rows), for each n-chunk (128 key rows):
      Lt[n, m]  = matmul(lhsT=T[:, n], rhs=T[:, m-pair])  (PSUM, A only)
      Et[n, m]  = exp(Lt)                                 (ACT, -> SBUF)
      y_psum   += Et.T @ x[n]            (MM3, accumulate over n)
      acc      += Et                     (row-sum accumulate, on DVE)
    rowsum[m] = reduce(transpose(acc))   (PE transpose + DVE reduce)
    y[m] = y_psum * (1 / rowsum)         (per-partition scale, DMA out)
  Row sums run off the PE (DVE accumulate + one transpose per m-block)
  because every extra matmul costs a ~188 ns fp32r weight load; the PE
  stays on the streaming floor (1 column/cycle), and every MM2 stream
  (213 ns) now exceeds the weight-load time so none of it is exposed.
  No max-subtraction in softmax: logits are O(1) here (|L| < ~4), and
  exp is computed in fp32. All matmuls run as float32r (TF32-mode,
  1 cyc/row at N>=256) with fp32 PSUM accumulation. The BIR verifier
  requires f32r matmul operands to be *produced* as f32r (rounding
  happens in the producing engine), hence the f32r-dtyped tiles and
  rounding copies.
"""

import numpy as np

S = 4096
D = 1024
R = 64
B = 8
P = 128
SC = S // P  # 32 s-chunks
DC = D // P  # 8 d-chunks
SG = 256     # phase-A s-group (2 chunks)
MG = 256     # main-loop m-group
NMG = S // MG


def build_bass():
    import concourse.bacc as bacc
    import concourse.mybir as mybir
    import concourse.tile as tile
    from concourse.masks import make_identity

    f32 = mybir.dt.float32
    f32r = mybir.dt.float32r

    nc = bacc.Bacc("TRN2", target_bir_lowering=False, debug=False)
    x_d = nc.dram_tensor("x", [S, D], f32, kind="ExternalInput").ap()
    q_d = nc.dram_tensor("q", [D, R], f32, kind="ExternalInput").ap()
    y_d = nc.dram_tensor("y", [S, D], f32, kind="ExternalOutput").ap()

    with tile.TileContext(nc) as tc:
        # ---- persistent pools ----
        with (
            tc.tile_pool(name="const", bufs=1) as cpool,
            tc.tile_pool(name="xres", bufs=1) as xpool,
            tc.tile_pool(name="tres", bufs=1) as tpool,
        ):
            ident = cpool.tile([P, P], f32, name="ident")
            make_identity(nc, ident)
            qs = cpool.tile([P, DC, P], f32r, name="qs")

            x_sb = xpool.tile([P, SC, D], f32r, name="x_sb")
            T_sb = tpool.tile([P, S], f32r, name="T_sb")

            # ---- phase A: load x, transpose, compute T = (x @ Qs)^T ----
            with (
                tc.tile_pool(name="pa_sbuf", bufs=2) as pa_pool,
                tc.tile_pool(name="pa_stage", bufs=7) as pa_stage,
                tc.tile_pool(name="pa_psum", bufs=3, space="PSUM") as pa_psum,
                tc.tile_pool(name="pa_tpsum", bufs=2, space="PSUM") as pa_tpsum,
            ):
                # qs padded to M=128 (cols R..127 zero) so MM1 writes all
                # 128 partitions of T and T needs no separate zeroing.
                qs_stage = pa_stage.tile([P, DC, P], f32, name="qs_stage", bufs=1)
                nc.vector.memset(qs_stage, 0.0)
                nc.sync.dma_start(
                    qs_stage[:, :, :R], q_d.rearrange("(dc p) r -> p dc r", p=P)
                )
                nc.vector.tensor_copy(qs[:], qs_stage[:])

                for g in range(S // SG):
                    stages = []
                    for s4 in range(SG // P):
                        sc = g * (SG // P) + s4
                        xstage = pa_stage.tile([P, D], f32, name="xstage")
                        nc.sync.dma_start(xstage[:], x_d[sc * P : (sc + 1) * P, :])
                        # off the critical path: ACT rounds x to f32r for MM3
                        nc.scalar.copy(x_sb[:, sc, :], xstage[:])
                        stages.append(xstage)
                    xT = pa_pool.tile([P, DC, SG], f32r, name="xT")
                    for dc in range(DC):
                        xTp = pa_psum.tile([P, SG], f32, name="xTp")
                        for s4 in range(SG // P):
                            nc.tensor.matmul(
                                xTp[:, s4 * P : (s4 + 1) * P],
                                stages[s4][:, dc * P : (dc + 1) * P],
                                ident,
                                is_transpose=True,
                                start=(s4 == 0),
                                stop=(s4 == SG // P - 1),
                            )
                        nc.vector.tensor_copy(xT[:, dc, :], xTp[:])
                    Tp = pa_tpsum.tile([P, SG], f32, name="Tp")
                    for dc in range(DC):
                        nc.tensor.matmul(
                            Tp[:],
                            qs[:, dc, :],
                            xT[:, dc, :],
                            start=(dc == 0),
                            stop=(dc == DC - 1),
                        )
                    nc.scalar.copy(T_sb[:, g * SG : (g + 1) * SG], Tp[:])

            # ---- main loop ----
            # m-groups are processed in pairs: during the even ("A") group's
            # n-loop, MM2 computes logits 512 wide (both halves of the pair)
            # and exp writes the A-half to a small rotating tile and the
            # B-half into a resident [P, SC, MG] buffer. The odd ("B")
            # group's n-loop then runs PV matmuls straight out of that
            # buffer with no MM2 at all. This halves MM2 weight loads and
            # keeps every MM2 stream (213 ns) longer than a weight load
            # (~187 ns), so no LDWEIGHTS time is exposed.
            with (
                tc.tile_pool(name="mn_sbuf", bufs=3) as mn_pool,
                tc.tile_pool(name="y_sbuf", bufs=3) as y_pool,
                tc.tile_pool(name="mn_psum", bufs=1, space="PSUM") as mn_psum,
                tc.tile_pool(name="lt_psum", bufs=2, space="PSUM") as lt_psum,
            ):
                NIT = NMG * SC
                ets = {}
                etB = mn_pool.tile([P, SC, MG], f32r, name="etB", bufs=1)

                def mm2_exp(k):
                    gm, n = divmod(k, SC)
                    assert gm % 2 == 0
                    m0 = gm * MG
                    ltp = lt_psum.tile([P, 2 * MG], f32, name="ltp", bufs=3)
                    nc.tensor.matmul(
                        ltp[:],
                        T_sb[:, n * P : (n + 1) * P],
                        T_sb[:, m0 : m0 + 2 * MG],
                        start=True,
                        stop=True,
                    )
                    etA = mn_pool.tile([P, MG], f32r, name="etA", bufs=4)
                    nc.scalar.activation(
                        etA[:], ltp[:, :MG], mybir.ActivationFunctionType.Exp
                    )
                    nc.scalar.activation(
                        etB[:, n, :], ltp[:, MG:], mybir.ActivationFunctionType.Exp
                    )
                    ets[k] = etA

                mm2_exp(0)
                mm2_exp(1)
                yp = acc = None
                for k in range(NIT):
                    gm, n = divmod(k, SC)
                    m0 = gm * MG
                    if k + 2 < NIT and (k + 2) // SC % 2 == 0:
                        mm2_exp(k + 2)
                    if n == 0:
                        yp = [
                            [
                                mn_psum.tile([P, 512], f32, name=f"yp_{mb}_{dh}")
                                for dh in range(2)
                            ]
                            for mb in range(2)
                        ]
                        acc = mn_pool.tile([P, MG], f32, name="acc", bufs=2)
                    et = ets.pop(k) if gm % 2 == 0 else etB[:, n, :]
                    for mb in range(2):
                        lhsT = et[:, mb * P : (mb + 1) * P]
                        for dh in range(2):
                            nc.tensor.matmul(
                                yp[mb][dh][:],
                                lhsT,
                                x_sb[:, n, dh * 512 : (dh + 1) * 512],
                                start=(n == 0),
                                stop=(n == SC - 1),
                            )
                    # row-sum accumulation off the PE: acc += et on DVE
                    if n == 0:
                        nc.vector.tensor_copy(acc[:], et[:])
                    else:
                        nc.vector.tensor_add(acc[:], acc[:], et[:])
                    if n == SC - 1:
                        # drain PSUM first (plain copies) so the next
                        # m-group's accumulating matmuls aren't blocked on
                        # the normalize chain; normalize in SBUF after.
                        y_sbs = []
                        for mb in range(2):
                            y_sb = y_pool.tile([P, D], f32, name="y_sb")
                            for dh in range(2):
                                nc.vector.tensor_copy(
                                    y_sb[:, dh * 512 : (dh + 1) * 512],
                                    yp[mb][dh][:],
                                )
                            y_sbs.append(y_sb)
                        for mb in range(2):
                            # acc holds colsums in [n-part, m]; transpose the
                            # mb block on the PE, reduce along free -> [m, 1]
                            accT = lt_psum.tile([P, P], f32, name="accT", bufs=1)
                            nc.tensor.matmul(
                                accT[:],
                                acc[:, mb * P : (mb + 1) * P],
                                ident,
                                is_transpose=True,
                                start=True,
                                stop=True,
                            )
                            rsum = mn_pool.tile([P, 1], f32, name="rsum")
                            nc.vector.reduce_sum(
                                rsum[:], accT[:], axis=mybir.AxisListType.X
                            )
                            inv = mn_pool.tile([P, 1], f32, name="inv")
                            nc.vector.reciprocal(inv[:], rsum[:])
                            y_sb = y_sbs[mb]
                            nc.vector.tensor_scalar_mul(y_sb[:], y_sb[:], inv[:])
                            r0 = m0 + mb * P
                            nc.sync.dma_start(y_d[r0 : r0 + P, :], y_sb[:])

    nc.compile()
    return nc


_NC_CACHE = None


def _get_nc():
    global _NC_CACHE
    if _NC_CACHE is None:
        _NC_CACHE = build_bass()
    return _NC_CACHE


def kernel(x: np.ndarray, Q: np.ndarray) -> np.ndarray:
    from concourse.bass_utils import run_bass_kernel_spmd

    x = np.asarray(x, dtype=np.float32)
    Q = np.asarray(Q, dtype=np.float32)
    assert x.shape == (B, S, D) and Q.shape == (D, R)
    qs = (Q * np.float32(1.0 / np.sqrt(D))).astype(np.float32)
    in_maps = [
        {"x": np.ascontiguousarray(x[b], dtype=np.float32), "q": qs} for b in range(B)
    ]
    nc = _get_nc()
    res = run_bass_kernel_spmd(nc, in_maps, core_ids=list(range(B)))
    out = np.stack([res.results[b]["y"] for b in range(B)], axis=0)
    return out.astype(np.float32)
